# revision 30
# baseline (speedup 1.0000x reference)
"""Data2VecVision self-attention Bass kernel for 8 Trainium2 NeuronCores.

Sharding: data-parallel over batch (64 = 8 cores x 8 batches/core).

Per-core design (v8 — fp8 DoubleRow Q/K projections):
  - hidden_states shard transposed on host to hsT [768, 8*197]; two SBUF
    copies: fp16 (V projection) and fp8e4 pair-packed st-major (Q/K).
  - Q/K projections are 3 DoubleRow fp8 matmuls each (K=256 logical per
    pass; measured ~2x fp16 throughput at FD=394). hs scaled x16, Wq/Wk
    x1024 into e4m3; descale folded into the PSUM->SBUF copies: Q on DVE
    (tensor_scalar dual-op: mult descale/8, add bq column), K on ACT
    (activation Copy with scale). Measured end-to-end rel err 1.83e-2
    (gate 2e-2, deterministic inputs).
  - 10 dummy matmuls on scratch SBUF at kernel start warm the PE HAM
    clock gate to 8/8 during the input-DMA head.
  - V stays fp16 ([s, d_out] natural layout, ones column for softmax
    sums, bv kept in V via the softmax identity).
  - scores per head in a 1-bank PSUM tile [128, 512] (per-head tiles
    keep the head-pair row-group matmuls concurrent, dt_start ~8ns); exp
    per head on ACT into a shared per-pair er tile; exp(bias) multiply
    per head on DVE (3/4) / GpSimd (1/4). All score-fronts of a 3-pair
    half emit before the ctx block (deep pipeline: the exp->mult chain
    gets a full ctx-block of slack).
  - context for 3 head-pairs accumulates into one 1-bank PSUM tile
    [128, 390]; normalization = DVE reciprocal + broadcast multiply ->
    fp16 staging; y stored fp16 (host casts back to fp32).
  - all input DRAM tensors are partition-major so every DMA moves
    multi-KB contiguous runs per partition; ordered so the first Q
    matmul unblocks after ~0.9 MB.
"""

import numpy as np
import ml_dtypes

import concourse.bacc as bacc
import concourse.mybir as mybir
import concourse.tile as tile
from concourse.bass_utils import run_bass_kernel_spmd

F32 = mybir.dt.float32
F16 = mybir.dt.float16
F8 = mybir.dt.float8e4
AF = mybir.ActivationFunctionType
ALU = mybir.AluOpType
DR = mybir.MatmulPerfMode.DoubleRow

N_CORES = 8
B = 64
NB = B // N_CORES          # batches per core
S = 197
HID = 768
HEADS = 12
D = 64
NHP = HEADS // 2           # head pairs
NCH = HID // 128           # 6 contraction chunks (fp16 view)
N_DR = 3                   # fp8 DoubleRow passes (256 hid dims each)
NST = 4                    # projection s-tiles per core
SW = NB * S // NST         # 394, projection moving width
CORE_S = NB * S            # 1576
JC = [(0, 128), (128, 69)]   # j/i chunk (offset, len)
QW = 1024                  # per-pair scores tile width (2 PSUM banks)

SC_HS = 16.0               # fp8 scale for hidden states
SC_W = 1024.0              # fp8 scale for Wq/Wk
DSC_Q = 1.0 / (SC_HS * SC_W * 8.0)   # descale + 1/sqrt(64)
DSC_K = 1.0 / (SC_HS * SC_W)


def _relative_position_index(h, w):
    coords = np.stack(np.meshgrid(np.arange(h), np.arange(w), indexing="ij")).reshape(2, -1)
    rel = coords[:, :, None] - coords[:, None, :]
    rel = rel.transpose(1, 2, 0).astype(np.int64)
    rel[:, :, 0] += h - 1
    rel[:, :, 1] += w - 1
    rel[:, :, 0] *= 2 * w - 1
    area = h * w
    nrd = (2 * h - 1) * (2 * w - 1) + 3
    idx = np.zeros((area + 1, area + 1), dtype=np.int64)
    idx[1:, 1:] = rel.sum(-1)
    idx[0, :] = nrd - 3
    idx[:, 0] = nrd - 2
    idx[0, 0] = nrd - 1
    return idx


def build_nc(reps=1):
    nc = bacc.Bacc("TRN2", target_bir_lowering=False, debug=False)

    SWB = N_DR * 2 * SW                      # 2364, hs8 bytes/partition per st
    hsT_d = nc.dram_tensor("hsT", [NCH, 128, CORE_S], F16, kind="ExternalInput").ap()
    hs8_d = nc.dram_tensor("hs8", [NST, 128, SWB], F8, kind="ExternalInput").ap()
    wq8_d = nc.dram_tensor("wq8", [128, NCH * N_DR * 256], F8, kind="ExternalInput").ap()
    wk8_d = nc.dram_tensor("wk8", [128, NCH * N_DR * 256], F8, kind="ExternalInput").ap()
    wv_d = nc.dram_tensor("wvT", [NCH, 128, HID], F16, kind="ExternalInput").ap()
    bq_d = nc.dram_tensor("bqc", [128, NCH], F32, kind="ExternalInput").ap()
    bv_d = nc.dram_tensor("bvb", [128, HID], F16, kind="ExternalInput").ap()
    eb_d = nc.dram_tensor("expb", [128, NHP * QW], F16, kind="ExternalInput").ap()
    y_d = nc.dram_tensor("y", [NB, S, HID], F16, kind="ExternalOutput").ap()

    with tile.TileContext(nc) as tc:
        with (
            tc.tile_pool(name="res", bufs=1) as res,
            tc.tile_pool(name="vpad", bufs=NB * 2) as vpad_pool,
            tc.tile_pool(name="et", bufs=6) as et_pool,
            tc.tile_pool(name="em", bufs=6) as em_pool,
            tc.tile_pool(name="rt", bufs=6) as rt_pool,
            tc.tile_pool(name="ot", bufs=6) as ot_pool,
            tc.tile_pool(name="pc", bufs=4, space="PSUM") as pc_ps,
            tc.tile_pool(name="sp", bufs=4, space="PSUM") as sc_ps,
        ):
            hs_sb = res.tile([128, NCH * CORE_S], F16)
            hs8_sb = res.tile([128, NST * SWB], F8)
            wq8_sb = res.tile([128, NCH * N_DR * 256], F8)
            wk8_sb = res.tile([128, NCH * N_DR * 256], F8)
            wv_sb = res.tile([128, NCH * HID], F16)
            bq_sb = res.tile([128, NCH], F32)
            bv_sb = res.tile([128, HID], F16)
            eb_sb = res.tile([128, NHP * QW], F16)
            qt_sb = res.tile([128, NCH * CORE_S], F16)
            kt_sb = res.tile([128, NCH * CORE_S + 64], F16)
            nc.vector.memset(kt_sb[:, NCH * CORE_S:], 0.0)
            vpad = [[vpad_pool.tile([128, HEADS * 65], F16, tag="vp",
                                    name=f"vpad_{b}_{j}") for j in range(2)]
                    for b in range(NB)]

            # moving-operand view of hs8: [p, st, h2, two, s]
            hs8_v = hs8_sb.rearrange("p (st h2 two s) -> p st h2 two s",
                                     st=NST, h2=N_DR, two=2)

            # scratch operands for the PE pre-warm (never initialized; the
            # results are never read). Warms the HAM clock gate to 8/8 during
            # the input-DMA head so real matmuls start at 2.4 GHz.
            warm_sb = res.tile([128, 640], F16)
            nc.vector.memset(warm_sb[:], 0.0)

            for _ in range(reps):
                wp = sc_ps.tile([128, 512], F32, tag="sp", name="warm_ps")
                for w in range(8):
                    nc.tensor.matmul(wp[:], warm_sb[:, :128], warm_sb[:, 128:640],
                                     start=True, stop=True)
                # ---- input DMAs (partition-major contiguous; ordered so the
                # first Q matmuls unblock early) ----
                dma_engs = [nc.sync, nc.scalar, nc.gpsimd]
                def dma(i, dst, src):
                    dma_engs[i % 3].dma_start(dst, src)
                # critical path: wq8 chunks c0-c2 then c3-c5 back-to-back on
                # sync (Q consumes c-chunks every ~0.5us); hs8 st0-2 on
                # scalar; bq+wk8+hs8st3 on gpsimd.
                CW = N_DR * 256
                dma(0, wq8_sb[:, :3 * CW], wq8_d[:, :3 * CW])
                dma(1, hs8_sb[:, 0:SWB], hs8_d[0])
                dma(2, bq_sb[:], bq_d[:])
                dma(0, wq8_sb[:, 3 * CW:], wq8_d[:, 3 * CW:])
                dma(2, wk8_sb[:], wk8_d[:])
                dma(1, hs8_sb[:, SWB:2 * SWB], hs8_d[1])
                dma(1, hs8_sb[:, 2 * SWB:3 * SWB], hs8_d[2])
                dma(2, hs8_sb[:, 3 * SWB:4 * SWB], hs8_d[3])
                # V inputs: first batches of attention order (6,7) first
                for c in range(NCH):
                    dma(c, hs_sb[:, c * CORE_S + 6 * S: (c + 1) * CORE_S],
                        hsT_d[c, :, 6 * S:])
                dma(0, wv_sb.rearrange("p (x e) -> p x e", e=HID),
                    wv_d.rearrange("x p e -> p x e"))
                dma(1, bv_sb[:], bv_d[:])
                for c in range(NCH):
                    dma(c + 1, hs_sb[:, c * CORE_S: c * CORE_S + 6 * S],
                        hsT_d[c, :, : 6 * S])
                dma(0, eb_sb[:], eb_d[:])

                # ---- Q/K projections, whole core ----
                def qk_mms(dst_ps, w8_sb, c, st):
                    for h2 in range(N_DR):
                        nc.tensor.matmul(
                            dst_ps[:],
                            w8_sb[:, (c * N_DR + h2) * 256:(c * N_DR + h2 + 1) * 256]
                            .rearrange("p (two m) -> p two m", two=2),
                            hs8_v[:, st, h2],
                            start=(h2 == 0), stop=(h2 == N_DR - 1),
                            perf_mode=DR)

                def emit_qk_st(st):
                    for c in range(NCH):
                        qp = pc_ps.tile([128, SW], F32, tag="pc")
                        qk_mms(qp, wq8_sb, c, st)
                        nc.vector.tensor_scalar(
                            qt_sb[:, c * CORE_S + st * SW: c * CORE_S + (st + 1) * SW],
                            qp[:], DSC_Q, bq_sb[:, c:c + 1], ALU.mult, ALU.add)
                    for c in range(NCH):
                        kp = pc_ps.tile([128, SW], F32, tag="pc")
                        qk_mms(kp, wk8_sb, c, st)
                        nc.scalar.activation(
                            kt_sb[:, c * CORE_S + st * SW: c * CORE_S + (st + 1) * SW],
                            kp[:], AF.Copy, scale=DSC_K)

                for st in range(NST):
                    emit_qk_st(st)

                # ---- V projection emitter ----
                def emit_v(b, jci, nts=(0, 1)):
                    joff, jlen = JC[jci]
                    vt = vpad[b][jci]
                    if 0 in nts:
                        ones_ap = vt[:jlen].rearrange("p (h c) -> p h c", h=HEADS)[:, :, 64:65]
                        nc.gpsimd.memset(ones_ap, 1.0)
                    scol = b * S + joff
                    for nt, (noff, nlen) in [(n, [(0, 512), (512, 256)][n]) for n in nts]:
                        vp = pc_ps.tile([128, 512], F32, tag="pc",
                                        name=f"vp_{b}_{jci}_{nt}")
                        for c in range(NCH):
                            nc.tensor.matmul(
                                vp[:jlen, :nlen],
                                hs_sb[:, c * CORE_S + scol: c * CORE_S + scol + jlen],
                                wv_sb[:, c * HID + noff: c * HID + noff + nlen],
                                start=(c == 0), stop=(c == NCH - 1))
                        dst = vt[:jlen, nt * 8 * 65:].rearrange(
                            "p (h c) -> p h c", c=65)[:, :nlen // 64, :64]
                        nc.vector.tensor_tensor(
                            out=dst, in0=vp[:jlen, :nlen],
                            in1=bv_sb[:jlen, noff:noff + nlen],
                            op=ALU.add)

                ATTN_ORDER = [6, 7, 0, 1, 2, 3, 4, 5]
                for jci in range(2):
                    emit_v(ATTN_ORDER[0], jci)

                # ---- attention (QK st-blocks for later batches interleaved
                # between attention batches: keeps the PE stream dense and
                # spreads ACT/DVE load across the whole kernel) ----
                for bk, b in enumerate(ATTN_ORDER):
                    nxt = ATTN_ORDER[bk + 1] if bk + 1 < NB else None
                    ot = [ot_pool.tile([128, HID], F16, tag="ot",
                                       name=f"ot_{b}_{i}") for i in range(2)]
                    for half in range(2):
                        cps = [pc_ps.tile([128, 390], F32, tag="pc",
                                          name=f"cp_{b}_{half}_{i}") for i in range(2)]

                        def emit_front(hp):
                            col = hp * CORE_S + b * S
                            er = et_pool.tile([128, QW], F16, tag="et",
                                              name=f"er_{b}_{hp}")
                            et = em_pool.tile([128, QW], F16, tag="em",
                                              name=f"em_{b}_{hp}")
                            for h in range(2):
                                sp = sc_ps.tile([128, 512], F32, tag="sp",
                                                name=f"sp_{b}_{hp}_{h}")
                                for jci in range(2):
                                    # jc1 reads a full 128-wide K slice (59 cols
                                    # of next-batch keys); those rows are zeroed
                                    # by the exp(bias) table.
                                    nc.tensor.matmul(
                                        sp[:, jci * S:(jci + 1) * S],
                                        kt_sb[h * 64:(h + 1) * 64,
                                              col + jci * 128: col + jci * 128 + 128],
                                        qt_sb[h * 64:(h + 1) * 64, col: col + S],
                                        start=True, stop=True)
                                nc.scalar.activation(
                                    er[:, h * 512: h * 512 + 2 * S],
                                    sp[:, : 2 * S], AF.Exp)
                                mul_eng = (nc.gpsimd if (h == 1 and hp % 2 == 0)
                                           else nc.vector)
                                mul_eng.tensor_tensor(
                                    out=et[:, h * 512: h * 512 + 2 * S],
                                    in0=er[:, h * 512: h * 512 + 2 * S],
                                    in1=eb_sb[:, hp * QW + h * 512:
                                              hp * QW + h * 512 + 2 * S],
                                    op=ALU.mult)
                            return et

                        def emit_ctx(hpl, et, ici):
                            ioff, ilen = JC[ici]
                            for h in range(2):
                                for jci, (joff, jlen) in enumerate(JC):
                                    nc.tensor.matmul(
                                        cps[ici][:ilen, hpl * 130 + h * 65:
                                                 hpl * 130 + (h + 1) * 65],
                                        et[:jlen, h * 512 + jci * S + ioff:
                                           h * 512 + jci * S + ioff + ilen],
                                        vpad[b][jci][:jlen,
                                                     ((half * 3 + hpl) * 2 + h) * 65:
                                                     ((half * 3 + hpl) * 2 + h + 1) * 65],
                                        start=(jci == 0), stop=(jci == 1))

                        ets = []
                        for hpl in range(3):
                            ets.append(emit_front(half * 3 + hpl))
                            if hpl == 1 and nxt is not None:
                                emit_v(nxt, half)
                        # ici-major: the ici=0 normalize+DMA overlaps ici=1's
                        # ctx matmuls (shorter tail, earlier PSUM free)
                        for ici, (ioff, ilen) in enumerate(JC):
                            for hpl in range(3):
                                emit_ctx(hpl, ets[hpl], ici)
                            r = rt_pool.tile([128, 6], F32, tag="rt",
                                             name=f"r_{b}_{half}_{ici}")
                            sums = cps[ici][:ilen].rearrange(
                                "p (g c) -> p g c", c=65)[:, :, 64:65]
                            nc.vector.reciprocal(r[:ilen], sums)
                            nc.vector.tensor_tensor(
                                out=ot[ici][:ilen, half * 384:(half + 1) * 384]
                                    .rearrange("p (g c) -> p g c", c=64),
                                in0=cps[ici][:ilen].rearrange(
                                    "p (g c) -> p g c", c=65)[:, :, :64],
                                in1=r[:ilen].broadcast_to([ilen, 6, 64]),
                                op=ALU.mult)
                            nc.sync.dma_start(
                                y_d[b, ioff:ioff + ilen, half * 384:(half + 1) * 384],
                                ot[ici][:ilen, half * 384:(half + 1) * 384])

    nc.compile()
    return nc


_NC_CACHE = {}


def _get_nc(reps=1):
    if reps not in _NC_CACHE:
        _NC_CACHE[reps] = build_nc(reps)
    return _NC_CACHE[reps]


def prep_inputs(hidden_states, Wq, bq, Wk, Wv, bv, bias_table):
    hidden_states = np.asarray(hidden_states, np.float32)
    Wq = np.asarray(Wq, np.float32)
    bq = np.asarray(bq, np.float32)
    Wk = np.asarray(Wk, np.float32)
    Wv = np.asarray(Wv, np.float32)
    bv = np.asarray(bv, np.float32)
    bias_table = np.asarray(bias_table, np.float32)
    E4 = ml_dtypes.float8_e4m3

    def w8_pack(wT):
        # wT [hid, dout] -> partition-major [128, (c h2 i m)]
        w = (wT * SC_W).astype(E4).astype(np.float32)
        w = w.reshape(N_DR, 2, 128, NCH, 128)       # [h2, i, k, c, m]
        w = w.transpose(2, 3, 0, 1, 4)               # [k, c, h2, i, m]
        return np.ascontiguousarray(w.reshape(128, NCH * N_DR * 256)).astype(E4)

    wq8 = w8_pack(Wq.T)
    wk8 = w8_pack(Wk.T)
    wvT = np.ascontiguousarray(Wv.T).reshape(NCH, 128, HID).astype(np.float16)
    bqc = np.ascontiguousarray((bq / 8.0).astype(np.float32).reshape(NCH, 128).T)
    bvb = np.ascontiguousarray(np.broadcast_to(bv, (128, HID))).astype(np.float16)

    idx = _relative_position_index(14, 14)
    bias_full = bias_table[idx]              # [S, S, HEADS] (i, j, h)
    biasT = bias_full.transpose(2, 1, 0)     # [h, j, i]
    # per-pair exp(bias) table [128, NHP*QW]: pair p, head h, jc quadrant at
    # h*512 + jc*197; gaps zero
    expb = np.zeros((128, NHP, QW), np.float32)
    for hp in range(NHP):
        for h in range(2):
            for jci, (joff, jlen) in enumerate(JC):
                expb[:jlen, hp, h * 512 + jci * S: h * 512 + (jci + 1) * S] = \
                    np.exp(biasT[2 * hp + h, joff:joff + jlen, :])
    expb = np.ascontiguousarray(expb.reshape(128, NHP * QW)).astype(np.float16)

    shared = {"wq8": wq8, "wk8": wk8, "wvT": wvT, "bqc": bqc, "bvb": bvb,
              "expb": expb}
    in_maps = []
    for cc in range(N_CORES):
        hs_c = hidden_states[cc * NB:(cc + 1) * NB]          # [NB, S, HID]
        hsT = np.ascontiguousarray(hs_c.transpose(2, 0, 1).reshape(HID, CORE_S))
        hs8 = (hsT * SC_HS).astype(E4)                        # [hid, CORE_S]
        # -> st-major [NST, 128(k), N_DR(h2), 2(i), SW]
        hs8 = hs8.reshape(N_DR, 2, 128, NST, SW).transpose(3, 2, 0, 1, 4)
        hs8 = np.ascontiguousarray(hs8.reshape(NST, 128, N_DR * 2 * SW))
        in_maps.append({"hsT": hsT.reshape(NCH, 128, CORE_S).astype(np.float16),
                        "hs8": hs8, **shared})
    return in_maps


def run(in_maps, reps=1, **kw):
    nc = _get_nc(reps)
    res = run_bass_kernel_spmd(nc, in_maps, core_ids=list(range(N_CORES)), **kw)
    out = np.concatenate([res.results[c]["y"] for c in range(N_CORES)], axis=0)
    return out.astype(np.float32), res


def kernel(hidden_states, Wq, bq, Wk, Wv, bv, bias_table,
           resolution_h=224, resolution_w=224):
    assert int(resolution_h) == 224 and int(resolution_w) == 224, \
        "kernel compiled for 224x224 (window 14x14, S=197)"
    hidden_states = np.asarray(hidden_states)
    assert hidden_states.shape == (B, S, HID), hidden_states.shape
    in_maps = prep_inputs(hidden_states, Wq, bq, Wk, Wv, bv, bias_table)
    return run(in_maps, reps=1)[0]


# revision 31
# speedup vs baseline: 1.0051x; 1.0051x over previous
"""Data2VecVision self-attention Bass kernel for 8 Trainium2 NeuronCores.

Sharding: data-parallel over batch (64 = 8 cores x 8 batches/core).

Per-core design (v8 — fp8 DoubleRow Q/K projections):
  - hidden_states shard transposed on host to hsT [768, 8*197]; two SBUF
    copies: fp16 (V projection) and fp8e4 pair-packed st-major (Q/K).
  - Q/K projections are 3 DoubleRow fp8 matmuls each (K=256 logical per
    pass; measured ~2x fp16 throughput at FD=394). hs scaled x16, Wq/Wk
    x1024 into e4m3; descale folded into the PSUM->SBUF copies: Q on DVE
    (tensor_scalar dual-op: mult descale/8, add bq column), K on ACT
    (activation Copy with scale). Measured end-to-end rel err 1.83e-2
    (gate 2e-2, deterministic inputs).
  - 10 dummy matmuls on scratch SBUF at kernel start warm the PE HAM
    clock gate to 8/8 during the input-DMA head.
  - V stays fp16 ([s, d_out] natural layout, ones column for softmax
    sums, bv kept in V via the softmax identity).
  - scores per head in a 1-bank PSUM tile [128, 512] (per-head tiles
    keep the head-pair row-group matmuls concurrent, dt_start ~8ns); exp
    per head on ACT into a shared per-pair er tile; exp(bias) multiply
    per head on DVE (3/4) / GpSimd (1/4). All score-fronts of a 3-pair
    half emit before the ctx block (deep pipeline: the exp->mult chain
    gets a full ctx-block of slack).
  - context for 3 head-pairs accumulates into one 1-bank PSUM tile
    [128, 390]; normalization = DVE reciprocal + broadcast multiply ->
    fp16 staging; y stored fp16 (host casts back to fp32).
  - all input DRAM tensors are partition-major so every DMA moves
    multi-KB contiguous runs per partition; ordered so the first Q
    matmul unblocks after ~0.9 MB.
"""

import numpy as np
import ml_dtypes

import concourse.bacc as bacc
import concourse.mybir as mybir
import concourse.tile as tile
from concourse.bass_utils import run_bass_kernel_spmd

F32 = mybir.dt.float32
F16 = mybir.dt.float16
F8 = mybir.dt.float8e4
AF = mybir.ActivationFunctionType
ALU = mybir.AluOpType
DR = mybir.MatmulPerfMode.DoubleRow

N_CORES = 8
B = 64
NB = B // N_CORES          # batches per core
S = 197
HID = 768
HEADS = 12
D = 64
NHP = HEADS // 2           # head pairs
NCH = HID // 128           # 6 contraction chunks (fp16 view)
N_DR = 3                   # fp8 DoubleRow passes (256 hid dims each)
NST = 4                    # projection s-tiles per core
SW = NB * S // NST         # 394, projection moving width
CORE_S = NB * S            # 1576
JC = [(0, 128), (128, 69)]   # j/i chunk (offset, len)
QW = 1024                  # per-pair scores tile width (2 PSUM banks)

SC_HS = 16.0               # fp8 scale for hidden states
SC_W = 1024.0              # fp8 scale for Wq/Wk
DSC_Q = 1.0 / (SC_HS * SC_W * 8.0)   # descale + 1/sqrt(64)
DSC_K = 1.0 / (SC_HS * SC_W)


def _relative_position_index(h, w):
    coords = np.stack(np.meshgrid(np.arange(h), np.arange(w), indexing="ij")).reshape(2, -1)
    rel = coords[:, :, None] - coords[:, None, :]
    rel = rel.transpose(1, 2, 0).astype(np.int64)
    rel[:, :, 0] += h - 1
    rel[:, :, 1] += w - 1
    rel[:, :, 0] *= 2 * w - 1
    area = h * w
    nrd = (2 * h - 1) * (2 * w - 1) + 3
    idx = np.zeros((area + 1, area + 1), dtype=np.int64)
    idx[1:, 1:] = rel.sum(-1)
    idx[0, :] = nrd - 3
    idx[:, 0] = nrd - 2
    idx[0, 0] = nrd - 1
    return idx


def build_nc(reps=1):
    nc = bacc.Bacc("TRN2", target_bir_lowering=False, debug=False)

    SWB = N_DR * 2 * SW                      # 2364, hs8 bytes/partition per st
    hsT_d = nc.dram_tensor("hsT", [NCH, 128, CORE_S], F16, kind="ExternalInput").ap()
    hs8_d = nc.dram_tensor("hs8", [NST, 128, SWB], F8, kind="ExternalInput").ap()
    wq8_d = nc.dram_tensor("wq8", [128, NCH * N_DR * 256], F8, kind="ExternalInput").ap()
    wk8_d = nc.dram_tensor("wk8", [128, NCH * N_DR * 256], F8, kind="ExternalInput").ap()
    wv_d = nc.dram_tensor("wvT", [NCH, 128, HID], F16, kind="ExternalInput").ap()
    bq_d = nc.dram_tensor("bqc", [128, NCH], F32, kind="ExternalInput").ap()
    bv_d = nc.dram_tensor("bvb", [128, HID], F16, kind="ExternalInput").ap()
    eb_d = nc.dram_tensor("expb", [128, NHP * QW], F16, kind="ExternalInput").ap()
    y_d = nc.dram_tensor("y", [NB, S, HID], F16, kind="ExternalOutput").ap()

    with tile.TileContext(nc) as tc:
        with (
            tc.tile_pool(name="res", bufs=1) as res,
            tc.tile_pool(name="vpad", bufs=NB * 2) as vpad_pool,
            tc.tile_pool(name="et", bufs=6) as et_pool,
            tc.tile_pool(name="em", bufs=6) as em_pool,
            tc.tile_pool(name="rt", bufs=6) as rt_pool,
            tc.tile_pool(name="ot", bufs=6) as ot_pool,
            tc.tile_pool(name="pc", bufs=4, space="PSUM") as pc_ps,
            tc.tile_pool(name="sp", bufs=4, space="PSUM") as sc_ps,
        ):
            hs_sb = res.tile([128, NCH * CORE_S], F16)
            hs8_sb = res.tile([128, NST * SWB], F8)
            wq8_sb = res.tile([128, NCH * N_DR * 256], F8)
            wk8_sb = res.tile([128, NCH * N_DR * 256], F8)
            wv_sb = res.tile([128, NCH * HID], F16)
            bq_sb = res.tile([128, NCH], F32)
            bv_sb = res.tile([128, HID], F16)
            eb_sb = res.tile([128, NHP * QW], F16)
            qt_sb = res.tile([128, NCH * CORE_S], F16)
            kt_sb = res.tile([128, NCH * CORE_S + 64], F16)
            nc.vector.memset(kt_sb[:, NCH * CORE_S:], 0.0)
            vpad = [[vpad_pool.tile([128, HEADS * 65], F16, tag="vp",
                                    name=f"vpad_{b}_{j}") for j in range(2)]
                    for b in range(NB)]

            # moving-operand view of hs8: [p, st, h2, two, s]
            hs8_v = hs8_sb.rearrange("p (st h2 two s) -> p st h2 two s",
                                     st=NST, h2=N_DR, two=2)

            # scratch operands for the PE pre-warm (never initialized; the
            # results are never read). Warms the HAM clock gate to 8/8 during
            # the input-DMA head so real matmuls start at 2.4 GHz.
            warm_sb = res.tile([128, 640], F16)
            nc.vector.memset(warm_sb[:], 0.0)

            for _ in range(reps):
                wp = sc_ps.tile([128, 512], F32, tag="sp", name="warm_ps")
                for w in range(8):
                    nc.tensor.matmul(wp[:], warm_sb[:, :128], warm_sb[:, 128:640],
                                     start=True, stop=True)
                # ---- input DMAs (partition-major contiguous; ordered so the
                # first Q matmuls unblock early) ----
                dma_engs = [nc.sync, nc.scalar, nc.gpsimd]
                def dma(i, dst, src):
                    dma_engs[i % 3].dma_start(dst, src)
                # critical path: wq8 chunks c0-c2 then c3-c5 back-to-back on
                # sync (Q consumes c-chunks every ~0.5us); hs8 st0-2 on
                # scalar; bq+wk8+hs8st3 on gpsimd.
                CW = N_DR * 256
                dma(0, wq8_sb[:, :3 * CW], wq8_d[:, :3 * CW])
                dma(1, hs8_sb[:, 0:SWB], hs8_d[0])
                dma(2, bq_sb[:], bq_d[:])
                dma(0, wq8_sb[:, 3 * CW:], wq8_d[:, 3 * CW:])
                dma(2, wk8_sb[:], wk8_d[:])
                dma(1, hs8_sb[:, SWB:2 * SWB], hs8_d[1])
                dma(1, hs8_sb[:, 2 * SWB:3 * SWB], hs8_d[2])
                dma(2, hs8_sb[:, 3 * SWB:4 * SWB], hs8_d[3])
                # V inputs: first batches of attention order (6,7) first
                for c in range(NCH):
                    dma(c, hs_sb[:, c * CORE_S + 6 * S: (c + 1) * CORE_S],
                        hsT_d[c, :, 6 * S:])
                dma(0, wv_sb.rearrange("p (x e) -> p x e", e=HID),
                    wv_d.rearrange("x p e -> p x e"))
                dma(1, bv_sb[:], bv_d[:])
                for c in range(NCH):
                    dma(c + 1, hs_sb[:, c * CORE_S: c * CORE_S + 6 * S],
                        hsT_d[c, :, : 6 * S])
                dma(0, eb_sb[:], eb_d[:])

                # ---- Q/K projections, whole core ----
                def qk_mms(dst_ps, w8_sb, c, st):
                    for h2 in range(N_DR):
                        nc.tensor.matmul(
                            dst_ps[:],
                            w8_sb[:, (c * N_DR + h2) * 256:(c * N_DR + h2 + 1) * 256]
                            .rearrange("p (two m) -> p two m", two=2),
                            hs8_v[:, st, h2],
                            start=(h2 == 0), stop=(h2 == N_DR - 1),
                            perf_mode=DR)

                def emit_qk_st(st):
                    for c in range(NCH):
                        qp = pc_ps.tile([128, SW], F32, tag="pc")
                        qk_mms(qp, wq8_sb, c, st)
                        nc.vector.tensor_scalar(
                            qt_sb[:, c * CORE_S + st * SW: c * CORE_S + (st + 1) * SW],
                            qp[:], DSC_Q, bq_sb[:, c:c + 1], ALU.mult, ALU.add)
                    for c in range(NCH):
                        kp = pc_ps.tile([128, SW], F32, tag="pc")
                        qk_mms(kp, wk8_sb, c, st)
                        nc.scalar.activation(
                            kt_sb[:, c * CORE_S + st * SW: c * CORE_S + (st + 1) * SW],
                            kp[:], AF.Copy, scale=DSC_K)

                for st in range(NST):
                    emit_qk_st(st)

                # ---- V projection emitter ----
                def emit_v(b, jci, nts=(0, 1)):
                    joff, jlen = JC[jci]
                    vt = vpad[b][jci]
                    if 0 in nts:
                        ones_ap = vt[:jlen].rearrange("p (h c) -> p h c", h=HEADS)[:, :, 64:65]
                        nc.gpsimd.memset(ones_ap, 1.0)
                    scol = b * S + joff
                    for nt, (noff, nlen) in [(n, [(0, 512), (512, 256)][n]) for n in nts]:
                        vp = pc_ps.tile([128, 512], F32, tag="pc",
                                        name=f"vp_{b}_{jci}_{nt}")
                        for c in range(NCH):
                            nc.tensor.matmul(
                                vp[:jlen, :nlen],
                                hs_sb[:, c * CORE_S + scol: c * CORE_S + scol + jlen],
                                wv_sb[:, c * HID + noff: c * HID + noff + nlen],
                                start=(c == 0), stop=(c == NCH - 1))
                        dst = vt[:jlen, nt * 8 * 65:].rearrange(
                            "p (h c) -> p h c", c=65)[:, :nlen // 64, :64]
                        nc.vector.tensor_tensor(
                            out=dst, in0=vp[:jlen, :nlen],
                            in1=bv_sb[:jlen, noff:noff + nlen],
                            op=ALU.add)

                ATTN_ORDER = [6, 7, 0, 1, 2, 3, 4, 5]
                for jci in range(2):
                    emit_v(ATTN_ORDER[0], jci)

                # ---- attention (QK st-blocks for later batches interleaved
                # between attention batches: keeps the PE stream dense and
                # spreads ACT/DVE load across the whole kernel) ----
                for bk, b in enumerate(ATTN_ORDER):
                    nxt = ATTN_ORDER[bk + 1] if bk + 1 < NB else None
                    ot = [ot_pool.tile([128, HID], F16, tag="ot",
                                       name=f"ot_{b}_{i}") for i in range(2)]
                    for half in range(2):
                        cps = [pc_ps.tile([128, 390], F32, tag="pc",
                                          name=f"cp_{b}_{half}_{i}") for i in range(2)]

                        def emit_front(hp):
                            col = hp * CORE_S + b * S
                            er = et_pool.tile([128, QW], F16, tag="et",
                                              name=f"er_{b}_{hp}")
                            et = em_pool.tile([128, QW], F16, tag="em",
                                              name=f"em_{b}_{hp}")
                            for h in range(2):
                                sp = sc_ps.tile([128, 512], F32, tag="sp",
                                                name=f"sp_{b}_{hp}_{h}")
                                for jci in range(2):
                                    # jc1 reads a full 128-wide K slice (59 cols
                                    # of next-batch keys); those rows are zeroed
                                    # by the exp(bias) table.
                                    nc.tensor.matmul(
                                        sp[:, jci * S:(jci + 1) * S],
                                        kt_sb[h * 64:(h + 1) * 64,
                                              col + jci * 128: col + jci * 128 + 128],
                                        qt_sb[h * 64:(h + 1) * 64, col: col + S],
                                        start=True, stop=True)
                                nc.scalar.activation(
                                    er[:, h * 512: h * 512 + 2 * S],
                                    sp[:, : 2 * S], AF.Exp)
                                mul_eng = (nc.gpsimd if (h == 1 and hp % 2 == 0)
                                           else nc.vector)
                                mul_eng.tensor_tensor(
                                    out=et[:, h * 512: h * 512 + 2 * S],
                                    in0=er[:, h * 512: h * 512 + 2 * S],
                                    in1=eb_sb[:, hp * QW + h * 512:
                                              hp * QW + h * 512 + 2 * S],
                                    op=ALU.mult)
                            return et

                        def emit_ctx(hpl, et):
                            for ici, (ioff, ilen) in enumerate(JC):
                                for h in range(2):
                                    for jci, (joff, jlen) in enumerate(JC):
                                        nc.tensor.matmul(
                                            cps[ici][:ilen, hpl * 130 + h * 65:
                                                     hpl * 130 + (h + 1) * 65],
                                            et[:jlen, h * 512 + jci * S + ioff:
                                               h * 512 + jci * S + ioff + ilen],
                                            vpad[b][jci][:jlen,
                                                         ((half * 3 + hpl) * 2 + h) * 65:
                                                         ((half * 3 + hpl) * 2 + h + 1) * 65],
                                            start=(jci == 0), stop=(jci == 1))

                        ets = []
                        for hpl in range(3):
                            ets.append(emit_front(half * 3 + hpl))
                            if hpl == 1 and nxt is not None:
                                emit_v(nxt, half)
                        for hpl in range(3):
                            emit_ctx(hpl, ets[hpl])

                        for ici, (ioff, ilen) in enumerate(JC):
                            r = rt_pool.tile([128, 6], F32, tag="rt",
                                             name=f"r_{b}_{half}_{ici}")
                            sums = cps[ici][:ilen].rearrange(
                                "p (g c) -> p g c", c=65)[:, :, 64:65]
                            nc.vector.reciprocal(r[:ilen], sums)
                            nc.vector.tensor_tensor(
                                out=ot[ici][:ilen, half * 384:(half + 1) * 384]
                                    .rearrange("p (g c) -> p g c", c=64),
                                in0=cps[ici][:ilen].rearrange(
                                    "p (g c) -> p g c", c=65)[:, :, :64],
                                in1=r[:ilen].broadcast_to([ilen, 6, 64]),
                                op=ALU.mult)
                            nc.sync.dma_start(
                                y_d[b, ioff:ioff + ilen, half * 384:(half + 1) * 384],
                                ot[ici][:ilen, half * 384:(half + 1) * 384])

    nc.compile()
    return nc


_NC_CACHE = {}


def _get_nc(reps=1):
    if reps not in _NC_CACHE:
        _NC_CACHE[reps] = build_nc(reps)
    return _NC_CACHE[reps]


def prep_inputs(hidden_states, Wq, bq, Wk, Wv, bv, bias_table):
    hidden_states = np.asarray(hidden_states, np.float32)
    Wq = np.asarray(Wq, np.float32)
    bq = np.asarray(bq, np.float32)
    Wk = np.asarray(Wk, np.float32)
    Wv = np.asarray(Wv, np.float32)
    bv = np.asarray(bv, np.float32)
    bias_table = np.asarray(bias_table, np.float32)
    E4 = ml_dtypes.float8_e4m3

    def w8_pack(wT):
        # wT [hid, dout] -> partition-major [128, (c h2 i m)]
        w = (wT * SC_W).astype(E4).astype(np.float32)
        w = w.reshape(N_DR, 2, 128, NCH, 128)       # [h2, i, k, c, m]
        w = w.transpose(2, 3, 0, 1, 4)               # [k, c, h2, i, m]
        return np.ascontiguousarray(w.reshape(128, NCH * N_DR * 256)).astype(E4)

    wq8 = w8_pack(Wq.T)
    wk8 = w8_pack(Wk.T)
    wvT = np.ascontiguousarray(Wv.T).reshape(NCH, 128, HID).astype(np.float16)
    bqc = np.ascontiguousarray((bq / 8.0).astype(np.float32).reshape(NCH, 128).T)
    bvb = np.ascontiguousarray(np.broadcast_to(bv, (128, HID))).astype(np.float16)

    idx = _relative_position_index(14, 14)
    bias_full = bias_table[idx]              # [S, S, HEADS] (i, j, h)
    biasT = bias_full.transpose(2, 1, 0)     # [h, j, i]
    # per-pair exp(bias) table [128, NHP*QW]: pair p, head h, jc quadrant at
    # h*512 + jc*197; gaps zero
    expb = np.zeros((128, NHP, QW), np.float32)
    for hp in range(NHP):
        for h in range(2):
            for jci, (joff, jlen) in enumerate(JC):
                expb[:jlen, hp, h * 512 + jci * S: h * 512 + (jci + 1) * S] = \
                    np.exp(biasT[2 * hp + h, joff:joff + jlen, :])
    expb = np.ascontiguousarray(expb.reshape(128, NHP * QW)).astype(np.float16)

    shared = {"wq8": wq8, "wk8": wk8, "wvT": wvT, "bqc": bqc, "bvb": bvb,
              "expb": expb}
    in_maps = []
    for cc in range(N_CORES):
        hs_c = hidden_states[cc * NB:(cc + 1) * NB]          # [NB, S, HID]
        hsT = np.ascontiguousarray(hs_c.transpose(2, 0, 1).reshape(HID, CORE_S))
        hs8 = (hsT * SC_HS).astype(E4)                        # [hid, CORE_S]
        # -> st-major [NST, 128(k), N_DR(h2), 2(i), SW]
        hs8 = hs8.reshape(N_DR, 2, 128, NST, SW).transpose(3, 2, 0, 1, 4)
        hs8 = np.ascontiguousarray(hs8.reshape(NST, 128, N_DR * 2 * SW))
        in_maps.append({"hsT": hsT.reshape(NCH, 128, CORE_S).astype(np.float16),
                        "hs8": hs8, **shared})
    return in_maps


def run(in_maps, reps=1, **kw):
    nc = _get_nc(reps)
    res = run_bass_kernel_spmd(nc, in_maps, core_ids=list(range(N_CORES)), **kw)
    out = np.concatenate([res.results[c]["y"] for c in range(N_CORES)], axis=0)
    return out.astype(np.float32), res


def kernel(hidden_states, Wq, bq, Wk, Wv, bv, bias_table,
           resolution_h=224, resolution_w=224):
    assert int(resolution_h) == 224 and int(resolution_w) == 224, \
        "kernel compiled for 224x224 (window 14x14, S=197)"
    hidden_states = np.asarray(hidden_states)
    assert hidden_states.shape == (B, S, HID), hidden_states.shape
    in_maps = prep_inputs(hidden_states, Wq, bq, Wk, Wv, bv, bias_table)
    return run(in_maps, reps=1)[0]
